# revision 9
# baseline (speedup 1.0000x reference)
"""Trainium2 Bass kernel for nn_ClusterLoss_Regr (topk_masking) — v2.

Computes  mean_b(128 - max_p((128 - d[b,p]) * [|proto[p] - label[b]| <= 0.5]))
for d: [8192, 4096] f32, labels: [8192] f32, proto: [4096] f32 -> scalar f32.

v2 design ("sorted staircase"):
  - Host sorts columns by proto value and rows by label.  Each row's mask is
    then one contiguous column range [lo_r, hi_r).  Rows are grouped into 64
    groups of 128 consecutive sorted rows; groups are rank-bucketed by their
    union mask width so the 8 groups of tile-index t have near-equal widths
    (uniform compile-time shapes across the 8 SPMD cores).  Columns outside
    a tile's union range are never staged or read (~21% HBM traffic cut).
  - Per 128-row tile the union range splits into an all-masked INTERIOR
    (every row of the tile wants these columns) and two narrow boundary
    BANDS where the mask varies by row.
  - INTERIOR: a custom single-source running-min DVE op with hand-written
    2X_1PORT / 2X_2PORT / 4X_2PORT uop programs -> 4 elem/lane/cycle.
    No mask needed, no proto data on device at all.
  - BANDS: the masked-min op compares an f16 index ramp against per-row
    range constants (|j - c0| <= 512 encodes j >= a_r or j < b_r), 2X mode.
    Mask bounds are computed EXACTLY on host (f32 predicate refinement).
  - All DVE ops write into one shared scratch buffer with descending end
    offsets (step 4) so every op's final running-min lands in a contiguous
    window -> ONE tail gather DMA instead of one per op.
  - d staged as bf16 (measured: plain bf16 HWDGE sustains ~395 GB/s/core;
    fp8 cast-DMA is write-side-bound and no faster).
Host: decode rowmins, map through the row permutation, mean in f64.
"""

import numpy as np

B, P = 8192, 4096
NCORES = 8
BSH = B // NCORES      # 1024 rows per core
RT = BSH // 128        # 8 row-tiles of 128 rows
NG = B // 128          # 64 row groups
ALIGN = 8
MAX_DIST = np.float32(128.0)
BIG = 2.0              # > max(d)=1.0
HALF_W = 512.0         # band compare halfwidth (band widths < 512)

_cache: dict = {}


def _ensure_path():
    try:
        import concourse.bass  # noqa: F401
    except ImportError:
        import sys

        for p in ("/opt/trn_rl_repo",):
            if p not in sys.path:
                sys.path.insert(0, p)


# --------------------------------------------------------------- DVE ops
def _build_maskmin_2x_uops():
    """2X_1PORT program for the masked-min op (see spec in _register_ops):
    v = select(C1 >= |Src1 - C0|, Src0, C2); acc = min(acc, v); acc streamed
    to both write halves.  Identical to the proven v1 program."""
    from concourse.dve_uop import (
        AluInp,
        AluOp,
        DelayInp,
        InpSel,
        OutPath,
        OutSel,
        Trigger,
        UopConfig,
        UopDpConfig,
    )

    ENABLE = 1
    P_AD = AluInp.PREV_ALU_OUT
    CUR = AluInp.CURR_ALU_OUT
    D = [
        AluInp.PREV_DELAY_0,
        AluInp.PREV_DELAY_1,
        AluInp.PREV_DELAY_2,
        AluInp.PREV_DELAY_3,
        AluInp.PREV_DELAY_4,
        AluInp.PREV_DELAY_5,
    ]
    SRC_DONE = (Trigger.SRC_TENSOR_DONE, Trigger.NONE, Trigger.NONE)
    COUNT_ONCE = (Trigger.COUNT, Trigger.NONE, Trigger.NONE)

    def wire_inputs(u):
        u.enable_input(InpSel.SRC_0, 0)      # input 0 -> b0's PREV_ALU_OUT
        u.enable_input(InpSel.SRC_1, 1)      # lane 0
        u.enable_input(InpSel.CONST_0, 2)    # lane 1: center
        u.enable_input(InpSel.CONST_1, 3)    # lane 2: halfwidth
        u.enable_input(InpSel.CONST_2, 4)    # lane 3: BIG
        u.enable_input(InpSel.SRC_0_HI, 5)   # lane 4
        u.enable_input(InpSel.SRC_1_HI, 6)   # lane 5

    def steady_blocks():
        dp = [UopDpConfig() for _ in range(8)]
        for i in range(8):
            dp[i].pass_through_delay(1, 2, 3, 5)
            if i not in (0, 3):
                dp[i].pass_through_delay(0)
            if i != 6:
                dp[i].pass_through_delay(4)
        dp[0].enable_alu(AluOp.ABSOLUTE_DIFF, D[0], D[1])
        dp[0].enable_delay_from_src(DelayInp.PREV_ALU_OUT, 0)  # Src0
        dp[1].enable_alu(AluOp.IS_GE, D[2], P_AD)
        dp[2].enable_alu(AluOp.SELECT, D[3], D[0])
        dp[3].enable_alu(AluOp.ABSOLUTE_DIFF, D[5], D[1])
        dp[3].enable_delay_from_src(DelayInp.PREV_ALU_OUT, 0)  # v_lo
        dp[4].enable_alu(AluOp.IS_GE, D[2], P_AD)
        dp[5].enable_alu(AluOp.SELECT, D[3], D[4])
        dp[6].enable_alu(AluOp.MIN, D[0], P_AD)
        dp[7].enable_alu(AluOp.MIN, CUR, P_AD)
        return dp

    sd = UopConfig(trigger=COUNT_ONCE, repeat_count=1, next_uop=(1, 0, 0))
    wire_inputs(sd)
    dp = steady_blocks()
    dp[7] = UopDpConfig()
    dp[7].pass_through_delay(0, 1, 2, 3, 4, 5)
    dp[7].enable_alu(AluOp.BYPASS, D[3], D[3])
    sd.datapath_config = dp

    st = UopConfig(trigger=SRC_DONE, require_inp0=ENABLE, require_inp1=ENABLE)
    wire_inputs(st)
    st.datapath_config = steady_blocks()
    st.enable_output(OutSel.ALU_OUT, OutPath.WR0_LO)
    st.enable_output(OutSel.ALU_OUT, OutPath.WR0_HI)
    return [sd, st]


def _build_scan_uops(nports):
    """Pure running-min programs.  nports=1 -> 2X_1PORT, 2 -> 2X_2PORT,
    4 -> 4X_2PORT.  Lanes: 0=SRC_0, 1=SRC_0_HI, 2=SRC_1, 3=SRC_1_HI,
    4=CONST_2 (seed).  state0 seeds blk7's out-flop; state1:
    acc = min(acc, min(elems)), streamed to the write ports."""
    from concourse.dve_uop import (
        AluInp,
        AluOp,
        DelayInp,
        InpSel,
        OutPath,
        OutSel,
        Trigger,
        UopConfig,
        UopDpConfig,
    )

    P_AD = AluInp.PREV_ALU_OUT
    CUR = AluInp.CURR_ALU_OUT
    D = [
        AluInp.PREV_DELAY_0,
        AluInp.PREV_DELAY_1,
        AluInp.PREV_DELAY_2,
        AluInp.PREV_DELAY_3,
        AluInp.PREV_DELAY_4,
        AluInp.PREV_DELAY_5,
    ]
    COUNT_ONCE = (Trigger.COUNT, Trigger.NONE, Trigger.NONE)
    SRC_DONE = (Trigger.SRC_TENSOR_DONE, Trigger.NONE, Trigger.NONE)

    def wire_inputs(u):
        u.enable_input(InpSel.SRC_0, 0)
        u.enable_input(InpSel.SRC_0_HI, 1)
        if nports >= 2:
            u.enable_input(InpSel.SRC_1, 2)
        if nports == 4:
            u.enable_input(InpSel.SRC_1_HI, 3)
        u.enable_input(InpSel.CONST_2, 4)

    def passthroughs():
        dp = [UopDpConfig() for _ in range(8)]
        for i in range(8):
            dp[i].pass_through_delay(0, 1, 2, 3, 4, 5)
        return dp

    def steady():
        dp = [UopDpConfig() for _ in range(8)]
        if nports == 4:
            dp[0].pass_through_delay(1, 2, 3, 4, 5)
            dp[0].enable_alu(AluOp.MIN, P_AD, D[0])        # m0 = min(A, B)
            dp[1].pass_through_delay(1, 2, 3, 4, 5)
            dp[1].enable_alu(AluOp.MIN, D[1], D[2])        # m1 = min(C, D)
            dp[1].enable_delay_from_src(DelayInp.PREV_ALU_OUT, 0)  # d0 <- m0
            dp[2].pass_through_delay(1, 2, 3, 4, 5)
            dp[2].enable_alu(AluOp.MIN, P_AD, D[0])        # m = min(m1, m0)
            first_bypass = 3
        elif nports == 2:
            dp[0].pass_through_delay(0, 1, 3, 4, 5)
            dp[0].enable_alu(AluOp.MIN, P_AD, D[1])        # min(A, C)
            first_bypass = 1
        else:
            dp[0].pass_through_delay(1, 2, 3, 4, 5)
            dp[0].enable_alu(AluOp.MIN, P_AD, D[0])        # min(A, B)
            first_bypass = 1
        for i in range(first_bypass, 7):
            dp[i].pass_through_delay(0, 1, 2, 3, 4, 5)
            dp[i].enable_alu(AluOp.BYPASS, P_AD, P_AD)
        dp[7].pass_through_delay(0, 1, 2, 3, 4, 5)
        dp[7].enable_alu(AluOp.MIN, CUR, P_AD)             # acc
        return dp

    sd = UopConfig(trigger=COUNT_ONCE, repeat_count=1, next_uop=(1, 0, 0))
    wire_inputs(sd)
    dp = passthroughs()
    dp[7] = UopDpConfig()
    dp[7].pass_through_delay(0, 1, 2, 4, 5)
    dp[7].enable_alu(AluOp.BYPASS, D[3], D[3])             # seed <- BIG
    sd.datapath_config = dp

    st = UopConfig(
        trigger=SRC_DONE,
        require_inp0=1,
        require_inp1=1 if nports >= 2 else 0,
    )
    wire_inputs(st)
    st.datapath_config = steady()
    st.enable_output(OutSel.ALU_OUT, OutPath.WR0_LO)
    st.enable_output(OutSel.ALU_OUT, OutPath.WR0_HI)
    if nports >= 2:
        st.enable_output(OutSel.ALU_OUT, OutPath.WR1_LO)
        st.enable_output(OutSel.ALU_OUT, OutPath.WR1_HI)
    return [sd, st]


def _register_ops():
    """Register the masked-min (2X) and pure-min-scan (4X) ops. Idempotent."""
    from concourse import dve_ops
    from concourse.dve_spec import (
        C0,
        C1,
        C2,
        AluOp,
        Bin,
        Spec,
        Src0,
        Src1,
        lower,
        scan,
        select,
    )
    from concourse.dve_uop import DveOpSpec

    def _make(name, spec, build_variants, perf_max, rd1_en):
        for op in dve_ops.OPS:
            if op.name == name:
                return op

        class _Op(dve_ops.DveOp):
            def compile(self, ver):
                key = (self.name, ver)
                if (r := dve_ops._COMPILE_CACHE.get(key)) is not None:
                    return r
                uops = lower(self.spec, ver=ver)
                variants = {}
                if ver == "v3":
                    variants = build_variants()
                    for v in variants.values():
                        assert len(v) == len(uops), (name, len(v), len(uops))
                result = DveOpSpec(
                    name=self.name,
                    opcode=dve_ops.get_dve_sub_opcode(self.name),
                    uops=uops,
                    uops_2x=variants.get("2x"),
                    uops_2x_2p=variants.get("2x_2p"),
                    uops_4x=variants.get("4x"),
                    perf_max=perf_max if variants else 0,
                    rd1_en=rd1_en,
                )
                dve_ops._COMPILE_CACHE[key] = result
                return result

        shas: dict = {}
        op = _Op(name, spec, subdim=False, uops_sha=shas)
        dve_ops.OPS.append(op)
        row = dve_ops._CUSTOM_DVE_ROW_BASE + len(dve_ops.OPS) - 1
        dve_ops._SUB_OPCODE_FOR_NAME[name] = row
        dve_ops.CUSTOM_DVE_SPECS[name] = spec
        for ver in ("v3", "v4"):
            shas[ver] = op.compile(ver).sha(ver) if ver == "v3" else ""
        return op

    def _mm_ref(in0, in1, s0, s1, imm2):
        mask = np.abs(in1.astype(np.float32) - np.asarray(s0, np.float32)) <= (
            np.float32(s1)
        )
        o = np.where(mask, in0.astype(np.float32), np.float32(imm2)).astype(
            np.float32
        )
        return np.minimum.accumulate(o, axis=-1)

    mm_spec = Spec(
        body=scan(
            AluOp.MIN,
            select(C1 >= Bin(AluOp.ABSOLUTE_DIFF, Src1, C0), Src0, C2),
            init=C2,
        ),
        reference=_mm_ref,
    )
    mm = _make(
        "CLUSTER_MASKMIN_ANT",
        mm_spec,
        lambda: {"2x": _build_maskmin_2x_uops()},
        perf_max=1,
        rd1_en=True,
    )

    def _sc_ref(in0, in1, s0, s1, imm2):
        o = np.minimum.accumulate(in0.astype(np.float32), axis=-1)
        return np.minimum(o, np.float32(imm2))

    sc_spec = Spec(body=scan(AluOp.MIN, Src0, init=C2), reference=_sc_ref)
    sc = _make(
        "PUREMIN_SCAN_ANT",
        sc_spec,
        lambda: {
            "2x": _build_scan_uops(1),
            "2x_2p": _build_scan_uops(2),
            "4x": _build_scan_uops(4),
        },
        perf_max=3,
        rd1_en=False,
    )
    return mm, sc


# --------------------------------------------------------------- host plan
def _exact_bounds(psort, lab):
    """Exact contiguous mask range per row: first/last sorted-proto index j
    with |f32(psort[j] - lab)| <= 0.5 (f32 predicate identical to the
    reference).  searchsorted gives a 1-ulp-accurate seed; refine locally."""
    lab = lab.astype(np.float32)
    n = len(psort)
    lo = np.searchsorted(psort, (lab - np.float32(0.5)).astype(np.float32),
                         side="left").astype(np.int64)
    hi = np.searchsorted(psort, (lab + np.float32(0.5)).astype(np.float32),
                         side="right").astype(np.int64)

    def pred(idx):
        idxc = np.clip(idx, 0, n - 1)
        v = np.abs((psort[idxc] - lab).astype(np.float32)) <= np.float32(0.5)
        return v & (idx >= 0) & (idx < n)

    for _ in range(3):
        lo = np.where(pred(lo - 1), lo - 1, lo)       # extend left
    for _ in range(3):
        shrink = ~pred(lo) & (lo < hi)
        lo = np.where(shrink, lo + 1, lo)             # shrink left
    for _ in range(3):
        hi = np.where(pred(hi), hi + 1, hi)           # extend right
    for _ in range(3):
        shrink = ~pred(hi - 1) & (hi > lo)
        hi = np.where(shrink, hi - 1, hi)             # shrink right
    hi = np.maximum(hi, lo)
    return lo, hi


def _plan(labels, proto):
    labels = np.asarray(labels, np.float32)
    proto = np.asarray(proto, np.float32)
    colperm = np.argsort(proto, kind="stable")
    psort = proto[colperm]
    roworder = np.argsort(labels, kind="stable")
    lo, hi = _exact_bounds(psort, labels[roworder])

    glo = lo.reshape(NG, 128)
    ghi = hi.reshape(NG, 128)
    gBLO = (glo.min(axis=1) // ALIGN) * ALIGN
    gBHI = -(-ghi.max(axis=1) // ALIGN) * ALIGN
    gW = np.maximum(gBHI - gBLO, ALIGN)

    rank = np.argsort(-gW, kind="stable")
    W_t = np.zeros(RT, np.int64)
    assign = np.zeros((NCORES, RT), np.int64)
    for t in range(RT):
        grp = rank[NCORES * t:NCORES * (t + 1)]
        W_t[t] = gW[grp].max()
        assign[:, t] = grp

    ILO_t = np.zeros(RT, np.int64)
    IHI_t = np.zeros(RT, np.int64)
    a_loc = np.zeros((NCORES, RT, 128), np.int64)
    b_loc = np.zeros((NCORES, RT, 128), np.int64)
    for t in range(RT):
        ilo, ihi = 0, 1 << 40
        for c in range(NCORES):
            g = assign[c, t]
            a = glo[g] - gBLO[g]
            b = ghi[g] - gBLO[g]
            a_loc[c, t] = a
            b_loc[c, t] = b
            ilo = max(ilo, -(-a.max() // ALIGN) * ALIGN)
            ihi = min(ihi, (b.min() // ALIGN) * ALIGN)
        ihi = min(ihi, int(W_t[t]))
        ilo = min(ilo, int(W_t[t]))
        if ihi < ilo:
            ihi = ilo
        ILO_t[t], IHI_t[t] = ilo, ihi

    # per-tile DMA split point (multiple of ALIGN, inside [ILO, IHI])
    MID_t = np.zeros(RT, np.int64)
    for t in range(RT):
        m = (int(W_t[t]) // 2 // ALIGN) * ALIGN
        MID_t[t] = min(max(m, int(ILO_t[t])), int(IHI_t[t]))
    # op list (half-gated order): (tile, kind, width, half)
    # kind 0=interior lo half, 3=interior hi half, 1=left band, 2=right band
    ops = []
    for t in range(RT):
        if MID_t[t] > ILO_t[t]:
            ops.append((t, 0, int(MID_t[t] - ILO_t[t]), 0))
        if IHI_t[t] > MID_t[t]:
            ops.append((t, 3, int(IHI_t[t] - MID_t[t]), 1))
        if ILO_t[t] > 0:
            ops.append((t, 1, int(ILO_t[t]), 0))
        if W_t[t] > IHI_t[t]:
            ops.append((t, 2, int(W_t[t] - IHI_t[t]), 1))
    rampw = max(
        [8] + [w for (_, k, w, _h) in ops if k in (1, 2)]
    )
    rampw = -(-rampw // ALIGN) * ALIGN
    return dict(colperm=colperm, roworder=roworder, gBLO=gBLO,
                W_t=W_t, assign=assign, ILO_t=ILO_t, IHI_t=IHI_t,
                MID_t=MID_t, a_loc=a_loc, b_loc=b_loc, ops=ops, rampw=rampw)


# --------------------------------------------------------------- device
def _get_bass(pl):
    key = ("v21", tuple(pl["W_t"]), tuple(pl["ILO_t"]), tuple(pl["IHI_t"]),
           tuple(pl["MID_t"]), pl["rampw"])
    if key in _cache:
        return _cache[key]
    _ensure_path()
    import concourse.bacc as bacc
    import concourse.mybir as mybir

    mm_op, sc_op = _register_ops()
    bf16 = mybir.dt.bfloat16
    f16 = mybir.dt.float16

    W_t = [int(x) for x in pl["W_t"]]
    ILO = [int(x) for x in pl["ILO_t"]]
    IHI = [int(x) for x in pl["IHI_t"]]
    MID = [int(x) for x in pl["MID_t"]]
    ops = pl["ops"]
    NOPS = len(ops)
    SUMW = sum(W_t)
    RAMPW = int(pl["rampw"])
    AUXW = RAMPW + 4 * RT  # trailing 2*RT f32 consts stored as f16 pairs
    MAXW = max(W_t)
    SCRW = MAXW + 4 * NOPS + 8

    nc = bacc.Bacc(
        "TRN2", target_bir_lowering=False, debug=False, num_devices=NCORES
    )
    d_ap = nc.dram_tensor("d", [128, SUMW], bf16, kind="ExternalInput").ap()
    aux_ap = nc.dram_tensor("aux", [128, AUXW], f16, kind="ExternalInput").ap()
    out_ap = nc.dram_tensor("rmin", [128, 4 * NOPS], bf16,
                            kind="ExternalOutput").ap()

    dbig = nc.alloc_sbuf_tensor("dbig_t", [128, SUMW], bf16).ap()
    scr = nc.alloc_sbuf_tensor("scr_t", [128, SCRW], bf16).ap()
    aux = nc.alloc_sbuf_tensor("aux_t", [128, AUXW], f16).ap()
    ramp = aux[:, :RAMPW]
    aux32 = aux.bitcast(mybir.dt.float32)  # [128, AUXW // 2]
    consL = aux32[:, RAMPW // 2:RAMPW // 2 + RT]
    consR = aux32[:, RAMPW // 2 + RT:RAMPW // 2 + 2 * RT]

    d_sems = [nc.alloc_semaphore(f"d{t}h{h}") for t in range(RT)
              for h in range(2)]
    aux_sem = nc.alloc_semaphore("aux")
    dve_sem = nc.alloc_semaphore("dve")
    out_sem = nc.alloc_semaphore("out")

    off_t = np.concatenate([[0], np.cumsum(W_t)])
    # DMA plan: (tile, half) halves split at MID; aux inserted after tile 1
    dma_list = []
    for t in range(RT):
        o = int(off_t[t])
        dma_list.append((t, 0, o, o + MID[t]))
        dma_list.append((t, 1, o + MID[t], o + W_t[t]))

    with nc.Block() as block:

        @block.sync
        def _(sync):
            sync.dma_start(aux[:], aux_ap[:]).then_inc(aux_sem, 16)
            for t, h, lo, hi in dma_list:
                if hi > lo:
                    # every op's gating half is nonempty by construction;
                    # an empty half has no waiter, so just skip its DMA
                    sync.dma_start(
                        dbig[:, lo:hi], d_ap[:, lo:hi]
                    ).then_inc(d_sems[2 * t + h], 16)
            # two-stage gather: op k's final lives at scr col SCRW-4k-1,
            # so early ops occupy the high end.  Bulk window (ops < split)
            # goes out once those retired; the last tile's ops go out in a
            # tiny final DMA.  out = concat(low, high) = scr tail, so the
            # host's column formula is unchanged.
            NTAIL = 4
            split = max(NOPS - NTAIL, 0)
            if split:
                sync.wait_ge(dve_sem, split)
                sync.dma_start(
                    out_ap[:, 4 * NTAIL:],
                    scr[:, SCRW - 4 * split:SCRW],
                ).then_inc(out_sem, 16)
            sync.wait_ge(dve_sem, NOPS)
            sync.dma_start(
                out_ap[:, :4 * NTAIL],
                scr[:, SCRW - 4 * NOPS:SCRW - 4 * split],
            ).then_inc(out_sem, 16)
            sync.wait_ge(out_sem, 32 if split else 16)
            all_sems = sorted(
                s.num for s in [*d_sems, aux_sem, dve_sem, out_sem]
            )
            lo = prev = all_sems[0]
            for n in all_sems[1:] + [None]:
                if n is not None and n == prev + 1:
                    prev = n
                    continue
                sync.sem_clear(range(lo, prev + 1))
                if n is not None:
                    lo = prev = n

        @block.vector
        def _(vector):
            waited_aux = False
            waited = set()
            for k, (t, kind, w, h) in enumerate(ops):
                if (t, h) not in waited:
                    vector.wait_ge(d_sems[2 * t + h], 16)
                    waited.add((t, h))
                if kind in (1, 2) and not waited_aux:
                    vector.wait_ge(aux_sem, 16)
                    waited_aux = True
                end = SCRW - 4 * k
                o = int(off_t[t])
                if kind == 0:
                    inst = nc.vector._custom_dve(
                        sc_op,
                        out=scr[:, end - w:end],
                        in0=dbig[:, o + ILO[t]:o + MID[t]],
                        imm2=float(BIG),
                    )
                    inst.ins.perf_max = 3
                elif kind == 3:
                    inst = nc.vector._custom_dve(
                        sc_op,
                        out=scr[:, end - w:end],
                        in0=dbig[:, o + MID[t]:o + IHI[t]],
                        imm2=float(BIG),
                    )
                    inst.ins.perf_max = 3
                elif kind == 1:
                    inst = nc.vector._custom_dve(
                        mm_op,
                        out=scr[:, end - w:end],
                        in0=dbig[:, o:o + w],
                        in1=ramp[:, :w],
                        s0=consL[:, t:t + 1],
                        s1=float(HALF_W),
                        imm2=float(BIG),
                    )
                    inst.ins.perf_max = 1
                else:
                    inst = nc.vector._custom_dve(
                        mm_op,
                        out=scr[:, end - w:end],
                        in0=dbig[:, o + IHI[t]:o + W_t[t]],
                        in1=ramp[:, :w],
                        s0=consR[:, t:t + 1],
                        s1=float(HALF_W),
                        imm2=float(BIG),
                    )
                    inst.ins.perf_max = 1
                inst.then_inc(dve_sem, 1)

    nc.compile()
    _cache[key] = nc
    return nc


# --------------------------------------------------------------- staging
def _stage(min_distances, labels, proto_classes, pl):
    import ml_dtypes

    bf16 = ml_dtypes.bfloat16
    d = np.asarray(min_distances, np.float32)
    dcols = np.ascontiguousarray(d[:, pl["colperm"]])
    W_t = pl["W_t"]
    rampw = int(pl["rampw"])
    ramp = np.arange(rampw, dtype=np.float16)

    in_maps = []
    for c in range(NCORES):
        segs = []
        aux = np.zeros((128, rampw + 4 * RT), np.float16)
        aux[:, :rampw] = ramp[None, :]
        consf32 = aux[:, rampw:].view(np.float32)  # [128, 2*RT]
        for t in range(RT):
            g = int(pl["assign"][c, t])
            rows = pl["roworder"][128 * g:128 * (g + 1)]
            blo = int(pl["gBLO"][g])
            w = int(W_t[t])
            seg = np.full((128, w), BIG, np.float32)
            real = max(0, min(w, P - blo))
            seg[:, :real] = dcols[rows, blo:blo + real]
            segs.append(seg)
            a = pl["a_loc"][c, t].astype(np.float32)
            bb = (pl["b_loc"][c, t] - pl["IHI_t"][t]).astype(np.float32)
            consf32[:, t] = a + np.float32(HALF_W)
            consf32[:, RT + t] = bb - np.float32(1.0) - np.float32(HALF_W)
        dcat = np.concatenate(segs, axis=1).astype(bf16)
        in_maps.append(
            {"d": np.ascontiguousarray(dcat), "aux": np.ascontiguousarray(aux)}
        )
    return in_maps


def kernel(min_distances, labels, proto_classes):
    _ensure_path()
    pl = _plan(labels, proto_classes)
    nc = _get_bass(pl)
    from concourse.bass_utils import run_bass_kernel_spmd

    in_maps = _stage(min_distances, labels, proto_classes, pl)
    res = run_bass_kernel_spmd(
        nc, in_maps, core_ids=list(range(NCORES))
    ).results

    ops = pl["ops"]
    NOPS = len(ops)
    loss_rows = np.zeros(B, np.float64)
    acc = np.full((NCORES, RT, 128), np.float32(BIG), np.float32)
    for c in range(NCORES):
        r = np.asarray(res[c]["rmin"]).astype(np.float32)  # [128, 4*NOPS]
        for k, (t, kind, w, h) in enumerate(ops):
            # op k's final value: scratch col (SCRW - 4k - 1) -> gather-local
            col = 4 * (NOPS - k) - 1
            acc[c, t] = np.minimum(acc[c, t], r[:, col])
    for c in range(NCORES):
        for t in range(RT):
            g = int(pl["assign"][c, t])
            rows = pl["roworder"][128 * g:128 * (g + 1)]
            dmin = acc[c, t]
            lr = np.where(
                dmin >= np.float32(BIG / 2),
                np.float32(128.0),
                (MAX_DIST - (MAX_DIST - dmin).astype(np.float32)).astype(
                    np.float32
                ),
            )
            loss_rows[rows] = lr
    return np.array(loss_rows.mean(dtype=np.float64), dtype=np.float32)
